# revision 1
# baseline (speedup 1.0000x reference)
"""Trainium2 Bass kernel for nn_GraphSemanticExtractor (GNN message passing).

Sharding (8 NeuronCores):
  Launch A: edge build        -- core c => (batch b=c//4, row-chunk rc=c%4 of 256 rows)
  Launch B: GAT layer 1       -- core c => (batch b=c//4, head hd=c%4)
  Launch C: GAT layer 2       -- same as B, inputs are B's per-head partial outputs
  Launch D: pool + proj head  -- core c => batch b=c (2 cores)

Key idea: the sparse top-k aggregation out[dst] += wgt*h[src] is done as a dense
matmul out.T = h.T @ R with R[s,t] = ew_k(s)*exp(lrelu(e_src[s]+e_dst[t])) at
t=topi[s,k].  R is built on the vector engine with iota-compare terms
(M0 = sum_k (iota==topi_k)*ew_k) and the attention factor applied densely.
Host-side work between launches is pure gather/transpose/concat glue.
"""

import sys

sys.path.insert(0, "/opt/trn_rl_repo")
sys.path.insert(0, "/opt/trn_rl_repo/concourse")

from contextlib import ExitStack

import ml_dtypes
import numpy as np

import concourse.bass as bass
import concourse.tile as tile
from concourse import bacc, mybir
from concourse.bass_utils import run_bass_kernel_spmd

F32 = mybir.dt.float32
BF16 = mybir.dt.bfloat16
U32 = mybir.dt.uint32
AF = mybir.ActivationFunctionType
OP = mybir.AluOpType
AX = mybir.AxisListType

B, S, H = 2, 1024, 1024
HEADS, K = 4, 8
SEM = 512
NB = H // 128  # 8 partition blocks
CH = S // 4    # 256 rows per edge-build core


def _mm_loop(ctx, nc, psum_pool, lhsT, rhs, mblocks, nsize, kblocks, evict):
    """out[m,n] = sum_k lhsT[k]^T rhs[k].  lhsT(k,m)->AP [128, Mblk], rhs(k,n)->AP [128,nn].
    evict(m, n0, nn, psum_ap) stores the [128, nn] f32 psum tile."""
    for m in range(mblocks):
        n0 = 0
        while n0 < nsize:
            nn = min(512, nsize - n0)
            pt = psum_pool.tile([128, nn], F32, tag="mmp")
            for k in range(kblocks):
                nc.tensor.matmul(
                    pt[:], lhsT(k, m), rhs(k, n0, nn),
                    start=(k == 0), stop=(k == kblocks - 1),
                )
            evict(m, n0, nn, pt[:])
            n0 += nn


def _build_A(nc):
    """Edge build: inputs xT (full, transposed), xTc (row chunk), phi_w.T, psi_w.T."""
    xT = nc.dram_tensor("xT", [H, S], F32, kind="ExternalInput")
    xTc = nc.dram_tensor("xTc", [H, CH], F32, kind="ExternalInput")
    pwT = nc.dram_tensor("pwT", [H, H], F32, kind="ExternalInput")
    swT = nc.dram_tensor("swT", [H, H], F32, kind="ExternalInput")
    srcx = nc.dram_tensor("srcx", [CH, 1], F32, kind="ExternalInput")
    topi = nc.dram_tensor("topi", [CH, K], U32, kind="ExternalOutput")
    ew = nc.dram_tensor("ew", [CH, K], F32, kind="ExternalOutput")

    with tile.TileContext(nc) as tc, ExitStack() as ctx:
        pers = ctx.enter_context(tc.tile_pool(name="pers", bufs=1))
        psum = ctx.enter_context(tc.tile_pool(name="psum", bufs=6, space="PSUM"))

        xT16 = pers.tile([128, NB, S], BF16, tag="xT16")
        xTc16 = pers.tile([128, NB, CH], BF16, tag="xTc16")
        pwT16 = pers.tile([128, NB, H], BF16, tag="pwT16")
        swT16 = pers.tile([128, NB, H], BF16, tag="swT16")
        xTr = xT[:].rearrange("(kb p) s -> p kb s", p=128)
        tmpa = ctx.enter_context(tc.tile_pool(name="tmpa", bufs=3))
        for kb in range(NB):
            stg = tmpa.tile([128, S], F32, tag="stg")
            nc.sync.dma_start(out=stg[:], in_=xTr[:, kb, :])
            nc.vector.tensor_copy(out=xT16[:, kb, :], in_=stg[:])
        nc.gpsimd.dma_start(out=xTc16[:], in_=xTc[:].rearrange("(kb p) s -> p kb s", p=128))
        nc.gpsimd.dma_start(out=pwT16[:], in_=pwT[:].rearrange("(kb p) s -> p kb s", p=128))
        nc.gpsimd.dma_start(out=swT16[:], in_=swT[:].rearrange("(kb p) s -> p kb s", p=128))

        psi16 = pers.tile([128, NB, S], BF16, tag="psi16")   # psi_h.T [e, t]
        phi16 = pers.tile([128, NB, CH], BF16, tag="phi16")  # phi_h.T [e, s-chunk]

        def ev_psi(m, n0, nn, pt):
            eng = nc.scalar if (m + n0) % 2 else nc.vector
            (eng.copy if eng is nc.scalar else eng.tensor_copy)(out=psi16[:, m, n0:n0 + nn], in_=pt)

        _mm_loop(ctx, nc, psum,
                 lambda k, m: swT16[:, k, m * 128:(m + 1) * 128],
                 lambda k, n0, nn: xT16[:, k, n0:n0 + nn],
                 NB, S, NB, ev_psi)

        def ev_phi(m, n0, nn, pt):
            nc.vector.tensor_copy(out=phi16[:, m, n0:n0 + nn], in_=pt)

        _mm_loop(ctx, nc, psum,
                 lambda k, m: pwT16[:, k, m * 128:(m + 1) * 128],
                 lambda k, n0, nn: xTc16[:, k, n0:n0 + nn],
                 NB, CH, NB, ev_phi)

        # scores [s-chunk, t] f32
        sc = pers.tile([128, 2, S], F32, tag="scores")

        def ev_sc(m, n0, nn, pt):
            nc.vector.tensor_copy(out=sc[:, m, n0:n0 + nn], in_=pt)

        _mm_loop(ctx, nc, psum,
                 lambda k, m: phi16[:, k, m * 128:(m + 1) * 128],
                 lambda k, n0, nn: psi16[:, k, n0:n0 + nn],
                 2, S, NB, ev_sc)

        # top-8 per row, softmax over the 8, self-edge mask
        mv = pers.tile([128, 2, K], F32, tag="mv")
        ti = pers.tile([128, 2, K], U32, tag="ti")
        for m in range(2):
            nc.vector.max(mv[:, m, :], sc[:, m, :])
            nc.vector.max_index(ti[:, m, :], mv[:, m, :], sc[:, m, :])
        ex = pers.tile([128, 2, K], F32, tag="ex")
        nc.scalar.activation(ex[:], mv[:], AF.Exp)
        sm = pers.tile([128, 2, 1], F32, tag="sm")
        nc.vector.tensor_reduce(sm[:], ex[:], axis=AX.X, op=OP.add)
        nc.vector.tensor_scalar(sm[:], sm[:], 1e-8, None, op0=OP.add)
        rc = pers.tile([128, 2, 1], F32, tag="rc")
        nc.vector.reciprocal(rc[:], sm[:])
        sx = pers.tile([128, 2, 1], F32, tag="sx")
        nc.sync.dma_start(out=sx[:], in_=srcx[:].rearrange("(m p) c -> p m c", p=128))
        tif = pers.tile([128, 2, K], F32, tag="tif")
        nc.vector.tensor_copy(out=tif[:], in_=ti[:])
        w8 = pers.tile([128, 2, K], F32, tag="w8")
        msk = pers.tile([128, 2, K], F32, tag="msk")
        for m in range(2):
            nc.vector.tensor_scalar(w8[:, m, :], ex[:, m, :], rc[:, m, :], 1e-8, op0=OP.mult, op1=OP.max)
            nc.vector.tensor_scalar(msk[:, m, :], tif[:, m, :], sx[:, m, :], None, op0=OP.is_equal)
            nc.vector.tensor_scalar(msk[:, m, :], msk[:, m, :], -1.0, 1.0, op0=OP.mult, op1=OP.add)
        ewt = pers.tile([128, 2, K], F32, tag="ewt")
        nc.vector.tensor_tensor(ewt[:], w8[:], msk[:], op=OP.mult)
        nc.sync.dma_start(out=topi[:].rearrange("(m p) k -> p m k", p=128), in_=ti[:])
        nc.sync.dma_start(out=ew[:].rearrange("(m p) k -> p m k", p=128), in_=ewt[:])
    nc.compile()
    return nc


def _build_BC(nc, first, skip_r=False, skip_hmm=False, skip_agg=False, skip_dma=False):
    """One GAT layer for one (batch, head).  Outputs gT[feat, node] = (agg/attn)/HEADS, bf16."""
    if first:
        xT = nc.dram_tensor("xT", [H, S], F32, kind="ExternalInput")
    else:
        ps = [nc.dram_tensor(f"p{i}", [H, S], BF16, kind="ExternalInput") for i in range(4)]
    WT = nc.dram_tensor("WT", [H, H], F32, kind="ExternalInput")
    a2r = nc.dram_tensor("a2r", [2, H], F32, kind="ExternalInput")
    tpf = nc.dram_tensor("tpf", [S, K], F32, kind="ExternalInput")
    tpi = nc.dram_tensor("tpi", [S, K], mybir.dt.int16, kind="ExternalInput")
    ewd = nc.dram_tensor("ewd", [S, K], F32, kind="ExternalInput")
    iot = nc.dram_tensor("iot", [1, S], F32, kind="ExternalInput")
    gT = nc.dram_tensor("gT", [H, S], BF16, kind="ExternalOutput")

    with tile.TileContext(nc) as tc, ExitStack() as ctx:
        pers = ctx.enter_context(tc.tile_pool(name="pers", bufs=1))
        tmp = ctx.enter_context(tc.tile_pool(name="tmp", bufs=3))
        psum = ctx.enter_context(tc.tile_pool(name="psum", bufs=5, space="PSUM"))
        psmall = ctx.enter_context(tc.tile_pool(name="psmall", bufs=1, space="PSUM"))

        xT16 = pers.tile([128, NB, S], BF16, tag="xT16")
        if first:
            nc.gpsimd.dma_start(out=xT16[:], in_=xT[:].rearrange("(kb p) s -> p kb s", p=128))
        else:
            for kb in range(NB):
                pin = [tmp.tile([128, S], BF16, tag=f"pin{i}", name=f"pin{i}") for i in range(4)]
                for i in range(4):
                    nc.sync.dma_start(
                        out=pin[i][:],
                        in_=ps[i][:].rearrange("(kb p) s -> p kb s", p=128)[:, kb, :])
                a01 = tmp.tile([128, S], BF16, tag="a01")
                a23 = tmp.tile([128, S], BF16, tag="a23")
                nc.vector.tensor_tensor(a01[:], pin[0][:], pin[1][:], op=OP.add)
                nc.vector.tensor_tensor(a23[:], pin[2][:], pin[3][:], op=OP.add)
                nc.vector.tensor_tensor(a01[:], a01[:], a23[:], op=OP.add)
                nc.scalar.activation(xT16[:, kb, :], a01[:], AF.Relu)

        WT16 = pers.tile([128, NB, H], BF16, tag="WT16")
        nc.gpsimd.dma_start(out=WT16[:], in_=WT[:].rearrange("(kb p) s -> p kb s", p=128))
        a2s = pers.tile([2, H], BF16, tag="a2s")
        nc.gpsimd.dma_start(out=a2s[:], in_=a2r[:])
        asb = pers.tile([128, H], BF16, tag="asb")
        adb = pers.tile([128, H], BF16, tag="adb")
        nc.gpsimd.partition_broadcast(asb[:], a2s[0:1, :])
        a2d1 = pers.tile([1, H], BF16, tag="a2d1")
        nc.sync.dma_start(out=a2d1[:], in_=a2s[1:2, :])
        nc.gpsimd.partition_broadcast(adb[:], a2d1[:])
        tpw = pers.tile([128, NB, K], mybir.dt.int16, tag="tpw")
        nc.sync.dma_start(out=tpw[:], in_=tpi[:].rearrange("(m p) k -> p m k", p=128))
        ews16 = pers.tile([128, NB, K], BF16, tag="ews16")
        nc.gpsimd.dma_start(out=ews16[:], in_=ewd[:].rearrange("(m p) k -> p m k", p=128))

        # h [node, feat] bf16
        h16 = pers.tile([128, NB, H], BF16, tag="h16")

        def ev_h(m, n0, nn, pt):
            eng = (m + n0 // 512) % 2
            if eng:
                nc.scalar.copy(out=h16[:, m, n0:n0 + nn], in_=pt)
            else:
                nc.vector.tensor_copy(out=h16[:, m, n0:n0 + nn], in_=pt)

        if skip_hmm:
            nc.vector.memset(h16[:], 0.0)
        else:
            _mm_loop(ctx, nc, psum,
                     lambda k, m: xT16[:, k, m * 128:(m + 1) * 128],
                     lambda k, n0, nn: WT16[:, k, n0:n0 + nn],
                     NB, H, NB, ev_h)

        # V = W^T [a_src|a_dst] -> [d, 2], via row-wise reductions of WT
        Vf = pers.tile([128, NB, 2], F32, tag="Vf")
        V16 = pers.tile([128, NB, 2], BF16, tag="V16")
        for m in range(NB):
            j1 = tmp.tile([128, H], BF16, tag="j1")
            nc.vector.scalar_tensor_tensor(j1[:], WT16[:, m, :], 1.0, asb[:],
                                           op0=OP.mult, op1=OP.mult,
                                           accum_out=Vf[:, m, 0:1])
            j2 = tmp.tile([128, H], BF16, tag="j2")
            nc.vector.scalar_tensor_tensor(j2[:], WT16[:, m, :], 1.0, adb[:],
                                           op0=OP.mult, op1=OP.mult,
                                           accum_out=Vf[:, m, 1:2])
        nc.vector.tensor_copy(out=V16[:], in_=Vf[:])

        # e_bothT [2, node] = V^T x
        ebT = pers.tile([2, S], F32, tag="ebT")

        def ev_e(m, n0, nn, pt):
            nc.vector.tensor_copy(out=ebT[:, n0:n0 + nn], in_=pt)

        for n0 in range(0, S, 512):
            pt = psmall.tile([2, 512], F32, tag="ebp")
            for k in range(NB):
                nc.tensor.matmul(pt[:], V16[:, k, :], xT16[:, k, n0:n0 + 512],
                                 start=(k == 0), stop=(k == NB - 1))
            ev_e(0, n0, 512, pt[:])

        edst1 = pers.tile([1, S], F32, tag="edst1")
        nc.sync.dma_start(out=edst1[:], in_=ebT[1:2, :])
        edb = pers.tile([128, S], F32, tag="edb")
        nc.gpsimd.partition_broadcast(edb[:], edst1[:])

        ones11 = pers.tile([1, 1], F32, tag="ones11")
        nc.vector.memset(ones11[:], 1.0)
        esc = pers.tile([128, NB, 1], F32, tag="esc")
        for m in range(NB):
            pt = psmall.tile([128, 1], F32, tag="escp")
            nc.tensor.matmul(pt[:], ebT[0:1, m * 128:(m + 1) * 128], ones11[:],
                             start=True, stop=True)
            nc.vector.tensor_copy(out=esc[:, m, :], in_=pt[:])

        # R [s, t] bf16: M0 = sum_k (iota==topi_k)*ew_k, then * exp(lrelu(e_src+e_dst))
        R = pers.tile([128, NB, S], BF16, tag="R")
        for m in range(0 if skip_r else NB):
            m0 = tmp.tile([128, S], BF16, tag="m0")
            nc.gpsimd.local_scatter(m0[:], ews16[:, m, :], tpw[:, m, :],
                                    channels=128, num_elems=S, num_idxs=K)
            zl = tmp.tile([128, S], F32, tag="zl")
            nc.scalar.activation(zl[:], edb[:], AF.Lrelu, bias=esc[:, m, :], alpha=0.2)
            ez = tmp.tile([128, S], BF16, tag="ez")
            nc.scalar.activation(ez[:], zl[:], AF.Exp)
            nc.vector.tensor_tensor(R[:, m, :], m0[:], ez[:], op=OP.mult)

        # attn^T [1, t] = 1^T R ; recip = 0.25 / (attn + 1e-8)
        onesc = pers.tile([128, 1], BF16, tag="onesc")
        nc.vector.memset(onesc[:], 1.0)
        atT = pers.tile([1, S], F32, tag="atT")
        for n0 in range(0, S, 512):
            pt = psmall.tile([1, 512], F32, tag="atp")
            for k in range(NB):
                nc.tensor.matmul(pt[:], onesc[:], R[:, k, n0:n0 + 512],
                                 start=(k == 0), stop=(k == NB - 1))
            nc.vector.tensor_copy(out=atT[:, n0:n0 + 512], in_=pt[:])
        nc.vector.tensor_scalar(atT[:], atT[:], 1e-8, None, op0=OP.add)
        arc = pers.tile([1, S], F32, tag="arc")
        nc.vector.reciprocal(arc[:], atT[:])
        nc.vector.tensor_scalar(arc[:], arc[:], 1.0 / HEADS, None, op0=OP.mult)
        rcb = pers.tile([128, S], F32, tag="rcb")
        nc.gpsimd.partition_broadcast(rcb[:], arc[:])

        # out^T [feat, t] = h^T R, scaled by rcb
        gsb = pers.tile([128, NB, S], BF16, tag="gsb")

        def ev_g(m, n0, nn, pt):
            nc.vector.tensor_tensor(gsb[:, m, n0:n0 + nn], pt, rcb[:, n0:n0 + nn], op=OP.mult)

        if skip_agg:
            nc.vector.memset(gsb[:], 0.0)
        else:
            _mm_loop(ctx, nc, psum,
                     lambda k, m: h16[:, k, m * 128:(m + 1) * 128],
                     lambda k, n0, nn: R[:, k, n0:n0 + nn],
                     NB, S, NB, ev_g)
        nc.sync.dma_start(out=gT[:].rearrange("(m p) t -> p m t", p=128), in_=gsb[:])
    nc.compile()
    return nc


def _build_D(nc):
    """x3 = relu(sum of per-head partials); attention pool over nodes; 2-layer head."""
    from concourse.masks import make_identity
    ps = [nc.dram_tensor(f"p{i}", [H, S], BF16, kind="ExternalInput") for i in range(4)]
    wpc = nc.dram_tensor("wpc", [H, 1], F32, kind="ExternalInput")
    w1T = nc.dram_tensor("w1T", [H, SEM], F32, kind="ExternalInput")
    b1c = nc.dram_tensor("b1c", [SEM, 1], F32, kind="ExternalInput")
    w2T = nc.dram_tensor("w2T", [SEM, SEM], F32, kind="ExternalInput")
    b2c = nc.dram_tensor("b2c", [SEM, 1], F32, kind="ExternalInput")
    res = nc.dram_tensor("res", [SEM, 1], F32, kind="ExternalOutput")

    with tile.TileContext(nc) as tc, ExitStack() as ctx:
        pers = ctx.enter_context(tc.tile_pool(name="pers", bufs=1))
        tmp = ctx.enter_context(tc.tile_pool(name="tmp", bufs=3))
        psum = ctx.enter_context(tc.tile_pool(name="psum", bufs=6, space="PSUM"))

        x3T = pers.tile([128, NB, S], BF16, tag="x3T")
        pt_ = [pers.tile([128, NB, S], BF16, tag=f"pin{i}", name=f"pin{i}") for i in range(4)]
        for i in range(4):
            nc.sync.dma_start(out=pt_[i][:], in_=ps[i][:].rearrange("(kb p) s -> p kb s", p=128))
        for kb in range(NB):
            a01 = tmp.tile([128, S], BF16, tag="a01")
            a23 = tmp.tile([128, S], BF16, tag="a23")
            nc.vector.tensor_tensor(a01[:], pt_[0][:, kb, :], pt_[1][:, kb, :], op=OP.add)
            nc.vector.tensor_tensor(a23[:], pt_[2][:, kb, :], pt_[3][:, kb, :], op=OP.add)
            nc.vector.tensor_tensor(a01[:], a01[:], a23[:], op=OP.add)
            nc.scalar.activation(x3T[:, kb, :], a01[:], AF.Relu)

        wp16 = pers.tile([128, NB, 1], BF16, tag="wp16")
        nc.gpsimd.dma_start(out=wp16[:], in_=wpc[:].rearrange("(kb p) c -> p kb c", p=128))
        psc = pers.tile([1, S], F32, tag="psc")
        for n0 in range(0, S, 512):
            pt = psum.tile([1, 512], F32, tag="sp")
            for k in range(NB):
                nc.tensor.matmul(pt[:], wp16[:, k, :], x3T[:, k, n0:n0 + 512],
                                 start=(k == 0), stop=(k == NB - 1))
            nc.vector.tensor_copy(out=psc[:, n0:n0 + 512], in_=pt[:])

        mx = pers.tile([1, 1], F32, tag="mx")
        nc.vector.tensor_reduce(mx[:], psc[:], axis=AX.X, op=OP.max)
        nmx = pers.tile([1, 1], F32, tag="nmx")
        nc.vector.tensor_scalar(nmx[:], mx[:], -1.0, None, op0=OP.mult)
        ev = pers.tile([1, S], F32, tag="ev")
        nc.scalar.activation(ev[:], psc[:], AF.Exp, bias=nmx[:])
        sm = pers.tile([1, 1], F32, tag="sm")
        nc.vector.tensor_reduce(sm[:], ev[:], axis=AX.X, op=OP.add)
        rc = pers.tile([1, 1], F32, tag="rc")
        nc.vector.reciprocal(rc[:], sm[:])
        alT = pers.tile([1, S], BF16, tag="alT")
        nc.vector.tensor_scalar(alT[:], ev[:], rc[:], None, op0=OP.mult)

        alb = pers.tile([128, S], BF16, tag="alb")
        nc.gpsimd.partition_broadcast(alb[:], alT[:])
        pldf = pers.tile([128, NB, 1], F32, tag="pldf")
        pld = pers.tile([128, NB, 1], BF16, tag="pld")
        for m in range(NB):
            junk = tmp.tile([128, S], BF16, tag="junk")
            nc.vector.scalar_tensor_tensor(junk[:], x3T[:, m, :], 1.0, alb[:],
                                           op0=OP.mult, op1=OP.mult,
                                           accum_out=pldf[:, m, :])
        nc.vector.tensor_copy(out=pld[:], in_=pldf[:])

        w116 = pers.tile([128, NB, SEM], BF16, tag="w116")
        nc.gpsimd.dma_start(out=w116[:], in_=w1T[:].rearrange("(kb p) c -> p kb c", p=128))
        b1f = pers.tile([128, 4, 1], F32, tag="b1f")
        nc.sync.dma_start(out=b1f[:], in_=b1c[:].rearrange("(m p) c -> p m c", p=128))
        hid = pers.tile([128, 4, 1], BF16, tag="hid")
        for m in range(4):
            pt = psum.tile([128, 1], F32, tag="sp")
            for k in range(NB):
                nc.tensor.matmul(pt[:], w116[:, k, m * 128:(m + 1) * 128], pld[:, k, :],
                                 start=(k == 0), stop=(k == NB - 1))
            nc.scalar.activation(hid[:, m, :], pt[:], AF.Relu, bias=b1f[:, m, :])

        w216 = pers.tile([128, 4, SEM], BF16, tag="w216")
        nc.gpsimd.dma_start(out=w216[:], in_=w2T[:].rearrange("(kb p) c -> p kb c", p=128))
        b2f = pers.tile([128, 4, 1], F32, tag="b2f")
        nc.sync.dma_start(out=b2f[:], in_=b2c[:].rearrange("(m p) c -> p m c", p=128))
        rsb = pers.tile([128, 4, 1], F32, tag="rsb")
        for m in range(4):
            pt = psum.tile([128, 1], F32, tag="sp")
            for k in range(4):
                nc.tensor.matmul(pt[:], w216[:, k, m * 128:(m + 1) * 128], hid[:, k, :],
                                 start=(k == 0), stop=(k == 3))
            nc.vector.tensor_tensor(rsb[:, m, :], pt[:], b2f[:, m, :], op=OP.add)
        nc.sync.dma_start(out=res[:].rearrange("(m p) c -> p m c", p=128), in_=rsb[:])
    nc.compile()
    return nc


_PROGS = {}


def _get_progs():
    if not _PROGS:
        def mk():
            return bacc.Bacc("TRN2", target_bir_lowering=False, debug=False,
                             enable_asserts=True, num_devices=8)
        _PROGS["A"] = _build_A(mk())
        _PROGS["B"] = _build_BC(mk(), first=True)
        _PROGS["C"] = _build_BC(mk(), first=False)
        _PROGS["D"] = _build_D(mk())
    return _PROGS


def kernel(hidden_states, phi_w, psi_w, gat_lin_w, gat_att, wp, w1, b1, w2, b2,
           _profile=None):
    f32 = np.float32
    bf16 = ml_dtypes.bfloat16
    hidden_states = np.asarray(hidden_states, f32)
    progs = _get_progs()
    C = lambda a: np.ascontiguousarray(a)
    times = {}

    def run(tag, in_maps, core_ids):
        r = run_bass_kernel_spmd(progs[tag], in_maps, core_ids=core_ids)
        if _profile is not None:
            times[tag] = r.exec_time_ns
        return r.results

    # ---- launch A: edge build ----
    xTb = [C(hidden_states[b].T) for b in range(B)]
    pwT, swT = C(np.asarray(phi_w, f32).T), C(np.asarray(psi_w, f32).T)
    in_a = []
    for c in range(8):
        b, rcn = c // 4, c % 4
        in_a.append({
            "xT": xTb[b], "xTc": C(xTb[b][:, rcn * CH:(rcn + 1) * CH]),
            "pwT": pwT, "swT": swT,
            "srcx": C(np.arange(rcn * CH, (rcn + 1) * CH, dtype=np.float32)[:, None]),
        })
    ra = run("A", in_a, list(range(8)))
    topi = np.stack([np.concatenate([ra[b * 4 + r]["topi"] for r in range(4)], 0) for b in range(B)])
    ew = np.stack([np.concatenate([ra[b * 4 + r]["ew"] for r in range(4)], 0) for b in range(B)])
    topi_f = topi.astype(f32)
    iota = np.arange(S, dtype=f32)[None, :]

    # ---- launches B, C: the two GAT layers ----
    ga = np.asarray(gat_att, f32)
    glw = np.asarray(gat_lin_w, f32)
    prev = None
    for li, tag in enumerate(("B", "C")):
        in_l = []
        for c in range(8):
            b, hd = c // 4, c % 4
            Wm = glw[li, hd * H:(hd + 1) * H, :]
            d = {
                "WT": C(Wm.T),
                "a2r": C(ga[li, hd].reshape(2, H)),
                "tpf": C(topi_f[b]), "tpi": C(topi[b].astype(np.int16)),
                "ewd": C(ew[b]), "iot": C(iota),
            }
            if li == 0:
                d["xT"] = xTb[b]
            else:
                for i in range(4):
                    d[f"p{i}"] = prev[b * 4 + i]
            in_l.append(d)
        rl = run(tag, in_l, list(range(8)))
        prev = [np.asarray(rl[c]["gT"], bf16) for c in range(8)]

    # ---- launch D: pooling + projection head ----
    in_d = []
    for b in range(B):
        d = {f"p{i}": prev[b * 4 + i] for i in range(4)}
        d.update({
            "wpc": C(np.asarray(wp, f32).reshape(H, 1)),
            "w1T": C(np.asarray(w1, f32).T), "b1c": C(np.asarray(b1, f32)[:, None]),
            "w2T": C(np.asarray(w2, f32).T), "b2c": C(np.asarray(b2, f32)[:, None]),
        })
        in_d.append(d)
    rd = run("D", in_d, [0, 1])
    out = np.stack([rd[b]["res"][:, 0].astype(f32) for b in range(B)])
    if _profile is not None:
        _profile.update(times)
    return out



# revision 14
# speedup vs baseline: 1.4289x; 1.4289x over previous
"""Trainium2 Bass kernel for nn_GraphSemanticExtractor (GNN message passing).

Sharding (8 NeuronCores), v3:
  Launch A:  edge build  -- core c => (batch b=c//4, row-chunk q=c%4); also
                            computes V = W^T[a_src|a_dst] for (l,hd)=(b,q).
  Launch B:  GAT layer 1 -- core c => (batch b=c//4, head hd=c%4)
  Launch C:  GAT layer 2 -- same mapping; inputs are B's per-head fp8 partials
  Launch D1: pooling     -- core c => (batch b=c//4, node-quarter q=c%4):
                            x3 = relu(sum/8), s=wp.x3, u=sum exp(s)x3, zeta
  Launch D2: head        -- core b (2 cores): pooled=sum u/sum zeta, w1/w2

Numerics: phi/psi/scores and h matmuls run in bf16 (input-quantization there
dominated the error budget); everything downstream is fp8: h is stored as
fp8 (4h), R[s,t] = 16*M0*exp(prelu(e_src+e_dst)) is fp8, the aggregation and
attn-sum run as fp8 DoubleRow matmuls, and per-head partials (2*out/attn)
travel between launches as fp8.  M0 = scatter(ew at topi) is built once in A
and shared by both layers; V rides along in A as a tiny fp8 DR matmul.
Consumers rebuild x_next = relu(sum/8) on the PE with identity-pair DR
matmuls straight from the four fp8 partials.
"""

import sys

sys.path.insert(0, "/opt/trn_rl_repo")
sys.path.insert(0, "/opt/trn_rl_repo/concourse")

from contextlib import ExitStack

import ml_dtypes
import numpy as np

import concourse.bass as bass
import concourse.tile as tile
from concourse import bacc, mybir
from concourse.bass_utils import run_bass_kernel_spmd

F32 = mybir.dt.float32
BF16 = mybir.dt.bfloat16
FP8 = mybir.dt.float8e4
I16 = mybir.dt.int16
U32 = mybir.dt.uint32
AF = mybir.ActivationFunctionType
OP = mybir.AluOpType
AX = mybir.AxisListType
DR = mybir.MatmulPerfMode.DoubleRow

B, S, H = 2, 1024, 1024
HEADS, K = 4, 8
SEM = 512
NB = H // 128   # 8 partition blocks
CH = S // 4     # 256 rows per edge-build / pooling core
WS = 64.0       # host-side weight scale for the (fp8) V path


def _ident2(nc, pers, tmp):
    """[128, 2, 128] fp8 identity pair for DR sums of 2 tensors."""
    from concourse.masks import make_identity
    id2 = pers.tile([128, 2, 128], FP8, tag="id2")
    idf = tmp.tile([128, 128], F32, tag="idf")
    make_identity(nc, idf)
    nc.vector.tensor_copy(out=id2[:, 0, :], in_=idf[:])
    nc.vector.tensor_copy(out=id2[:, 1, :], in_=idf[:])
    return id2


def _build_A(nc):
    """Edge build (bf16) + V projections (fp8)."""
    xT16 = nc.dram_tensor("xT16", [H, S], BF16, kind="ExternalInput")
    xTq16 = nc.dram_tensor("xTq16", [H, CH], BF16, kind="ExternalInput")
    pwT16 = nc.dram_tensor("pwT16", [H, H], BF16, kind="ExternalInput")
    swT16 = nc.dram_tensor("swT16", [H, H], BF16, kind="ExternalInput")
    Wr8 = nc.dram_tensor("Wr8", [H, H], FP8, kind="ExternalInput")
    a2w8 = nc.dram_tensor("a2w8", [128, NB * 2], FP8, kind="ExternalInput")
    srcx = nc.dram_tensor("srcx", [CH, 1], F32, kind="ExternalInput")
    m0o = nc.dram_tensor("m0o", [CH, S], BF16, kind="ExternalOutput")
    V16o = nc.dram_tensor("V16o", [128, NB * 2], BF16, kind="ExternalOutput")

    with tile.TileContext(nc) as tc, ExitStack() as ctx:
        pers = ctx.enter_context(tc.tile_pool(name="pers", bufs=1))
        psum = ctx.enter_context(tc.tile_pool(name="psum", bufs=6, space="PSUM"))
        psmall = ctx.enter_context(tc.tile_pool(name="psmall", bufs=2, space="PSUM"))

        x16 = pers.tile([128, NB, S], BF16, tag="x16")
        xq16 = pers.tile([128, NB, CH], BF16, tag="xq16")
        pw16 = pers.tile([128, NB, H], BF16, tag="pw16")
        sw16 = pers.tile([128, NB, H], BF16, tag="sw16")
        wr8 = pers.tile([128, NB, H], FP8, tag="wr8")
        aw8 = pers.tile([128, NB, 2], FP8, tag="aw8")
        nc.sync.dma_start(out=x16[:], in_=xT16[:].rearrange("(kb p) s -> p kb s", p=128))
        nc.scalar.dma_start(out=sw16[:], in_=swT16[:].rearrange("(kb p) s -> p kb s", p=128))
        nc.scalar.dma_start(out=pw16[:], in_=pwT16[:].rearrange("(kb p) s -> p kb s", p=128))
        nc.sync.dma_start(out=xq16[:], in_=xTq16[:].rearrange("(kb p) s -> p kb s", p=128))
        nc.gpsimd.dma_start(out=wr8[:], in_=Wr8[:].rearrange("(kb p) s -> p kb s", p=128))
        nc.gpsimd.dma_start(out=aw8[:], in_=a2w8[:].rearrange("p (kb j) -> p kb j", kb=NB))
        sx = pers.tile([128, 2, 1], F32, tag="sx")
        nc.sync.dma_start(out=sx[:], in_=srcx[:].rearrange("(m p) c -> p m c", p=128))

        # V = W^T a (fp8 x64 each) -> psum = 4096 V -> evict bf16 /16 -> 256 V
        Vf = pers.tile([128, NB, 2], BF16, tag="Vf")
        for m in range(NB):
            pv = psmall.tile([128, 2], F32, tag="pv", name="pv")
            for kp in range(NB // 2):
                nc.tensor.matmul(
                    pv[:], wr8[:, 2 * kp:2 * kp + 2, m * 128:(m + 1) * 128],
                    aw8[:, 2 * kp:2 * kp + 2, :],
                    start=(kp == 0), stop=(kp == NB // 2 - 1), perf_mode=DR)
            nc.vector.tensor_scalar(Vf[:, m, :], pv[:], 1.0 / 16.0, None, op0=OP.mult)
        nc.sync.dma_start(out=V16o[:].rearrange("p (kb j) -> p kb j", kb=NB), in_=Vf[:])

        # psi.T [e, t] for all t (bf16)
        psi16 = pers.tile([128, NB, S], BF16, tag="psi16")
        for m in range(NB):
            for half in range(2):
                pt = psum.tile([128, 512], F32, tag="mmp")
                cs = slice(half * 512, (half + 1) * 512)
                for k in range(NB):
                    nc.tensor.matmul(pt[:], sw16[:, k, m * 128:(m + 1) * 128],
                                     x16[:, k, cs],
                                     start=(k == 0), stop=(k == NB - 1))
                if (m + half) % 2:
                    nc.scalar.copy(psi16[:, m, cs], pt[:])
                else:
                    nc.vector.tensor_copy(out=psi16[:, m, cs], in_=pt[:])

        # phi.T [e, s-chunk] (bf16)
        phi16 = pers.tile([128, NB, CH], BF16, tag="phi16")
        for m in range(NB):
            pt = psum.tile([128, 512], F32, tag="mmp")
            for k in range(NB):
                nc.tensor.matmul(pt[:, 0:CH], pw16[:, k, m * 128:(m + 1) * 128],
                                 xq16[:, k, :],
                                 start=(k == 0), stop=(k == NB - 1))
            nc.scalar.copy(phi16[:, m, :], pt[:, 0:CH])

        # scores chunk [CH, S] -> top8 -> softmax -> mask -> scatter, per m
        m0t = pers.tile([128, 2, S], BF16, tag="m0t")
        mv = pers.tile([128, 2, K], F32, tag="mv")
        ti = pers.tile([128, 2, K], U32, tag="ti")
        ex = pers.tile([128, 2, K], F32, tag="ex")
        sm = pers.tile([128, 2, 1], F32, tag="sm")
        rc = pers.tile([128, 2, 1], F32, tag="rc")
        tif = pers.tile([128, 2, K], F32, tag="tif")
        tiw = pers.tile([128, 2, K], I16, tag="tiw")
        w8 = pers.tile([128, 2, K], F32, tag="w8")
        msk = pers.tile([128, 2, K], F32, tag="msk")
        ewt = pers.tile([128, 2, K], BF16, tag="ewt")
        sc = pers.tile([128, 2, S], F32, tag="sc")
        for m in range(2):
            for half in range(2):
                pt = psum.tile([128, 512], F32, tag="mmp")
                cs = slice(half * 512, (half + 1) * 512)
                for k in range(NB):
                    nc.tensor.matmul(
                        pt[:], phi16[:, k, m * 128:(m + 1) * 128],
                        psi16[:, k, cs],
                        start=(k == 0), stop=(k == NB - 1))
                nc.vector.tensor_copy(out=sc[:, m, cs], in_=pt[:])
            nc.vector.max(mv[:, m, :], sc[:, m, :])
            nc.vector.max_index(ti[:, m, :], mv[:, m, :], sc[:, m, :])
            nc.scalar.activation(ex[:, m, :], mv[:, m, :], AF.Exp)
            nc.vector.tensor_reduce(sm[:, m, :], ex[:, m, :], axis=AX.X, op=OP.add)
            nc.vector.tensor_scalar(sm[:, m, :], sm[:, m, :], 1e-8, None, op0=OP.add)
            nc.vector.reciprocal(rc[:, m, :], sm[:, m, :])
            nc.vector.tensor_copy(out=tif[:, m, :], in_=ti[:, m, :])
            nc.vector.tensor_copy(out=tiw[:, m, :], in_=ti[:, m, :])
            nc.vector.tensor_scalar(w8[:, m, :], ex[:, m, :], rc[:, m, :], 1e-8,
                                    op0=OP.mult, op1=OP.max)
            nc.vector.tensor_scalar(msk[:, m, :], tif[:, m, :], sx[:, m, :], None,
                                    op0=OP.is_equal)
            nc.vector.tensor_scalar(msk[:, m, :], msk[:, m, :], -1.0, 1.0,
                                    op0=OP.mult, op1=OP.add)
            nc.vector.tensor_tensor(ewt[:, m, :], w8[:, m, :], msk[:, m, :],
                                    op=OP.mult)
            nc.gpsimd.local_scatter(m0t[:, m, :], ewt[:, m, :], tiw[:, m, :],
                                    channels=128, num_elems=S, num_idxs=K)
        nc.sync.dma_start(out=m0o[:].rearrange("(m p) t -> p m t", p=128), in_=m0t[:])
    nc.compile()
    return nc


def _build_BC(nc, first):
    """One GAT layer for one (batch, head).  Emits fp8 partial 2*out/attn."""
    if first:
        xT16 = nc.dram_tensor("xT16", [H, S], BF16, kind="ExternalInput")
    else:
        ps = [nc.dram_tensor(f"p{i}", [H, S], FP8, kind="ExternalInput")
              for i in range(4)]
    WT16 = nc.dram_tensor("WT16", [H, H], BF16, kind="ExternalInput")
    m0d = nc.dram_tensor("m0d", [S, S], BF16, kind="ExternalInput")
    V16 = nc.dram_tensor("V16", [128, NB * 2], BF16, kind="ExternalInput")
    gT = nc.dram_tensor("gT", [H, S], FP8, kind="ExternalOutput")

    with tile.TileContext(nc) as tc, ExitStack() as ctx:
        pers = ctx.enter_context(tc.tile_pool(name="pers", bufs=1))
        tmp = ctx.enter_context(tc.tile_pool(name="tmp", bufs=4))
        psum = ctx.enter_context(tc.tile_pool(name="psum", bufs=4, space="PSUM"))
        psmall = ctx.enter_context(tc.tile_pool(name="psmall", bufs=2, space="PSUM"))

        vf = pers.tile([128, NB, 2], BF16, tag="vf")
        nc.sync.dma_start(out=vf[:], in_=V16[:].rearrange("p (kb j) -> p kb j", kb=NB))
        m016 = pers.tile([128, NB, S], BF16, tag="m016")
        nc.gpsimd.dma_start(out=m016[:], in_=m0d[:].rearrange("(kb p) s -> p kb s", p=128))
        w16 = pers.tile([128, NB, H], BF16, tag="w16")
        nc.scalar.dma_start(out=w16[:], in_=WT16[:].rearrange("(kb p) s -> p kb s", p=128))

        # x tile (bf16): layer1 = x; layer2 = relu(sum(partials)/8)
        x16 = pers.tile([128, NB, S], BF16, tag="x16t")
        if first:
            nc.sync.dma_start(out=x16[:], in_=xT16[:].rearrange("(kb p) s -> p kb s", p=128))
        else:
            pin = pers.tile([128, 4, NB, S], FP8, tag="pin")
            for i in range(4):
                eng = [nc.sync, nc.scalar, nc.gpsimd, nc.sync][i]
                eng.dma_start(out=pin[:, i, :, :],
                              in_=ps[i][:].rearrange("(kb p) s -> p kb s", p=128))
            id2 = _ident2(nc, pers, tmp)
            for kb in range(NB):
                for half in range(2):
                    pt = psum.tile([128, 512], F32, tag="mmp")
                    cs = slice(half * 512, (half + 1) * 512)
                    nc.tensor.matmul(pt[:], id2[:], pin[:, 0:2, kb, cs],
                                     start=True, stop=False, perf_mode=DR)
                    nc.tensor.matmul(pt[:], id2[:], pin[:, 2:4, kb, cs],
                                     start=False, stop=True, perf_mode=DR)
                    nc.scalar.activation(x16[:, kb, cs], pt[:], AF.Relu, scale=0.125)

        # e_src per-partition [128, NB, 1] and e_dstT [1, S] (bf16 matmuls)
        esc = pers.tile([128, NB, 1], F32, tag="esc")
        for m in range(NB):
            pv = psmall.tile([128, 2], F32, tag="pv", name="pv")[:, 0:1]
            for k in range(NB):
                nc.tensor.matmul(pv, x16[:, k, m * 128:(m + 1) * 128],
                                 vf[:, k, 0:1],
                                 start=(k == 0), stop=(k == NB - 1))
            nc.vector.tensor_scalar(esc[:, m, :], pv, 1.0 / 256.0, None, op0=OP.mult)
        edT = pers.tile([1, S], F32, tag="edT")
        for half in range(2):
            pv = psum.tile([128, 512], F32, tag="mmp", name="pv1")[0:1, :]
            cs = slice(half * 512, (half + 1) * 512)
            for k in range(NB):
                nc.tensor.matmul(pv, vf[:, k, 1:2], x16[:, k, cs],
                                 start=(k == 0), stop=(k == NB - 1))
            nc.vector.tensor_scalar(edT[:, cs], pv, 1.0 / 256.0, None, op0=OP.mult)
        edb = pers.tile([128, S], F32, tag="edb")
        nc.gpsimd.partition_broadcast(edb[:], edT[:])

        # R8 = 16 * M0 * exp(prelu(e_src + e_dst))  (fp8) -- on Act + DVE,
        # runs concurrently with the h matmul below (different engines).
        R8 = pers.tile([128, NB, S], FP8, tag="R8")
        for kb in range(NB):
            zl = tmp.tile([128, S], F32, tag="zl")
            nc.scalar.activation(zl[:], edb[:], AF.Prelu, bias=esc[:, kb, :],
                                 alpha=0.2)
            ez = tmp.tile([128, S], BF16, tag="ez")
            nc.scalar.activation(ez[:], zl[:], AF.Exp)
            nc.vector.scalar_tensor_tensor(R8[:, kb, :], m016[:, kb, :], 16.0,
                                           ez[:], op0=OP.mult, op1=OP.mult)

        # h (bf16) -> h8 = 4h fp8; attn' DR matmuls interleaved into the loop
        h8 = pers.tile([128, NB, H], FP8, tag="h8")
        ones2t = pers.tile([128, 2, 16], FP8, tag="ones2t")
        nc.vector.memset(ones2t[:], 1.0)
        ones2 = ones2t[:, :, 0:1]
        atp = [psmall.tile([128, 512], F32, tag="atp", name=f"atp{h_}")[0:1, :]
               for h_ in range(2)]
        for m in range(NB):
            for half in range(2):
                pt = psum.tile([128, 512], F32, tag="mmp")
                cs = slice(half * 512, (half + 1) * 512)
                for k in range(NB):
                    nc.tensor.matmul(pt[:], x16[:, k, m * 128:(m + 1) * 128],
                                     w16[:, k, cs],
                                     start=(k == 0), stop=(k == NB - 1))
                if (m + half) % 2:
                    nc.scalar.mul(h8[:, m, cs], pt[:], 4.0)
                else:
                    nc.vector.tensor_scalar(h8[:, m, cs], pt[:], 4.0, None,
                                            op0=OP.mult)
            if m < 4:  # attn kp=m needs R8 blocks 2m, 2m+1
                for half in range(2):
                    cs = slice(half * 512, (half + 1) * 512)
                    nc.tensor.matmul(atp[half], ones2, R8[:, 2 * m:2 * m + 2, cs],
                                     start=(m == 0), stop=(m == 3), perf_mode=DR)

        atT = pers.tile([1, S], F32, tag="atT")
        for half in range(2):
            cs = slice(half * 512, (half + 1) * 512)
            nc.vector.tensor_scalar(atT[:, cs], atp[half], 1.6e-7, None, op0=OP.add)
        arc = pers.tile([1, S], F32, tag="arc")
        nc.vector.reciprocal(arc[:], atT[:])
        nc.vector.tensor_scalar(arc[:], arc[:], 0.5, None, op0=OP.mult)
        rcb = pers.tile([128, S], F32, tag="rcb")
        nc.gpsimd.partition_broadcast(rcb[:], arc[:])

        # agg: gsb[m, t] = (sum_s h8[s, m*128:] R8[s, t]) * rcb[t]  (fp8 out)
        gsb = pers.tile([128, NB, S], FP8, tag="gsb")
        for half in range(2):
            cs = slice(half * 512, (half + 1) * 512)
            for mg in range(2):
                pts = [psum.tile([128, 512], F32, tag="mmp", name=f"aggp{i}")
                       for i in range(4)]
                for kp in range(NB // 2):
                    for mi, pt in enumerate(pts):
                        m = mg * 4 + mi
                        nc.tensor.matmul(
                            pt[:], h8[:, 2 * kp:2 * kp + 2, m * 128:(m + 1) * 128],
                            R8[:, 2 * kp:2 * kp + 2, cs],
                            start=(kp == 0), stop=(kp == NB // 2 - 1), perf_mode=DR)
                for mi, pt in enumerate(pts):
                    m = mg * 4 + mi
                    nc.vector.tensor_tensor(gsb[:, m, cs], pt[:], rcb[:, cs],
                                            op=OP.mult)
        nc.sync.dma_start(out=gT[:].rearrange("(m p) t -> p m t", p=128), in_=gsb[:])
    nc.compile()
    return nc


def _build_D1(nc):
    """Pooling partials for one (batch, node-quarter):
    x3q = relu(sum/8); s = wp.x3q; u = sum_t e^s x3q[:,t]; zeta = sum e^s."""
    ps = [nc.dram_tensor(f"p{i}", [H, CH], FP8, kind="ExternalInput")
          for i in range(4)]
    wpc = nc.dram_tensor("wpc", [H, 1], BF16, kind="ExternalInput")
    uo = nc.dram_tensor("uo", [H, 1], F32, kind="ExternalOutput")
    zo = nc.dram_tensor("zo", [1, 1], F32, kind="ExternalOutput")

    with tile.TileContext(nc) as tc, ExitStack() as ctx:
        pers = ctx.enter_context(tc.tile_pool(name="pers", bufs=1))
        tmp = ctx.enter_context(tc.tile_pool(name="tmp", bufs=3))
        psum = ctx.enter_context(tc.tile_pool(name="psum", bufs=6, space="PSUM"))

        pin = pers.tile([128, 4, NB, CH], FP8, tag="pin")
        for i in range(4):
            eng = [nc.sync, nc.scalar, nc.gpsimd, nc.sync][i]
            eng.dma_start(out=pin[:, i, :, :],
                          in_=ps[i][:].rearrange("(kb p) s -> p kb s", p=128))
        wp16 = pers.tile([128, NB, 1], BF16, tag="wp16")
        nc.gpsimd.dma_start(out=wp16[:], in_=wpc[:].rearrange("(kb p) c -> p kb c", p=128))

        id2 = _ident2(nc, pers, tmp)
        x3 = pers.tile([128, NB, CH], BF16, tag="x3")
        for kb in range(NB):
            pt = psum.tile([128, 512], F32, tag="mmp")
            nc.tensor.matmul(pt[:, 0:CH], id2[:], pin[:, 0:2, kb, :],
                             start=True, stop=False, perf_mode=DR)
            nc.tensor.matmul(pt[:, 0:CH], id2[:], pin[:, 2:4, kb, :],
                             start=False, stop=True, perf_mode=DR)
            nc.scalar.activation(x3[:, kb, :], pt[:, 0:CH], AF.Relu, scale=0.125)

        psc = pers.tile([1, CH], F32, tag="psc")
        pv = psum.tile([128, 512], F32, tag="mmp", name="pv1")[0:1, 0:CH]
        for k in range(NB):
            nc.tensor.matmul(pv, wp16[:, k, :], x3[:, k, :],
                             start=(k == 0), stop=(k == NB - 1))
        nc.vector.tensor_copy(out=psc[:], in_=pv)

        evw = pers.tile([1, CH], F32, tag="evw")
        nc.scalar.activation(evw[:], psc[:], AF.Exp)
        zeta = pers.tile([1, 1], F32, tag="zeta")
        nc.vector.tensor_reduce(zeta[:], evw[:], axis=AX.X, op=OP.add)
        ew16 = pers.tile([1, CH], BF16, tag="ew16")
        nc.vector.tensor_copy(out=ew16[:], in_=evw[:])
        alb = pers.tile([128, CH], BF16, tag="alb")
        nc.gpsimd.partition_broadcast(alb[:], ew16[:])

        uf = pers.tile([128, NB, 1], F32, tag="uf")
        for m in range(NB):
            junk = tmp.tile([128, CH], BF16, tag="junk")
            nc.vector.scalar_tensor_tensor(junk[:], x3[:, m, :], 1.0, alb[:],
                                           op0=OP.mult, op1=OP.mult,
                                           accum_out=uf[:, m, :])
        nc.sync.dma_start(out=uo[:].rearrange("(kb p) c -> p kb c", p=128), in_=uf[:])
        nc.sync.dma_start(out=zo[:], in_=zeta[:])
    nc.compile()
    return nc


def _build_D2(nc):
    """Combine pooling partials and run the projection head (per batch)."""
    uq = nc.dram_tensor("uq", [H, 4], F32, kind="ExternalInput")
    zq = nc.dram_tensor("zq", [1, 4], F32, kind="ExternalInput")
    w1T = nc.dram_tensor("w1T", [H, SEM], BF16, kind="ExternalInput")
    b1c = nc.dram_tensor("b1c", [SEM, 1], F32, kind="ExternalInput")
    w2T = nc.dram_tensor("w2T", [SEM, SEM], BF16, kind="ExternalInput")
    b2c = nc.dram_tensor("b2c", [SEM, 1], F32, kind="ExternalInput")
    res = nc.dram_tensor("res", [SEM, 1], F32, kind="ExternalOutput")

    with tile.TileContext(nc) as tc, ExitStack() as ctx:
        pers = ctx.enter_context(tc.tile_pool(name="pers", bufs=1))
        psmall = ctx.enter_context(tc.tile_pool(name="psmall", bufs=2, space="PSUM"))

        u4 = pers.tile([128, NB, 4], F32, tag="u4")
        nc.sync.dma_start(out=u4[:], in_=uq[:].rearrange("(kb p) c -> p kb c", p=128))
        z4 = pers.tile([1, 4], F32, tag="z4")
        nc.sync.dma_start(out=z4[:], in_=zq[:])
        w116 = pers.tile([128, NB, SEM], BF16, tag="w116")
        nc.gpsimd.dma_start(out=w116[:], in_=w1T[:].rearrange("(kb p) c -> p kb c", p=128))
        b1f = pers.tile([128, 4, 1], F32, tag="b1f")
        nc.sync.dma_start(out=b1f[:], in_=b1c[:].rearrange("(m p) c -> p m c", p=128))
        w216 = pers.tile([128, 4, SEM], BF16, tag="w216")
        nc.gpsimd.dma_start(out=w216[:], in_=w2T[:].rearrange("(kb p) c -> p kb c", p=128))
        b2f = pers.tile([128, 4, 1], F32, tag="b2f")
        nc.sync.dma_start(out=b2f[:], in_=b2c[:].rearrange("(m p) c -> p m c", p=128))

        zs = pers.tile([1, 1], F32, tag="zs")
        nc.vector.tensor_reduce(zs[:], z4[:], axis=AX.X, op=OP.add)
        zr = pers.tile([1, 1], F32, tag="zr")
        nc.vector.reciprocal(zr[:], zs[:])
        zrb = pers.tile([128, 1], F32, tag="zrb")
        nc.gpsimd.partition_broadcast(zrb[:], zr[:])
        usum = pers.tile([128, NB, 1], F32, tag="usum")
        nc.vector.tensor_reduce(usum[:], u4[:], axis=AX.X, op=OP.add)
        pld = pers.tile([128, NB, 1], BF16, tag="pld")
        nc.vector.tensor_scalar(pld[:], usum[:], zrb[:, 0:1], None, op0=OP.mult)

        hid = pers.tile([128, 4, 1], BF16, tag="hid")
        for m in range(4):
            pv = psmall.tile([128, 2], F32, tag="pv", name="pv")[:, 0:1]
            for k in range(NB):
                nc.tensor.matmul(pv, w116[:, k, m * 128:(m + 1) * 128],
                                 pld[:, k, :], start=(k == 0), stop=(k == NB - 1))
            nc.scalar.activation(hid[:, m, :], pv, AF.Relu, bias=b1f[:, m, :])
        rsb = pers.tile([128, 4, 1], F32, tag="rsb")
        for m in range(4):
            pv = psmall.tile([128, 2], F32, tag="pv", name="pv")[:, 0:1]
            for k in range(4):
                nc.tensor.matmul(pv, w216[:, k, m * 128:(m + 1) * 128],
                                 hid[:, k, :], start=(k == 0), stop=(k == 3))
            nc.vector.tensor_tensor(rsb[:, m, :], pv, b2f[:, m, :], op=OP.add)
        nc.sync.dma_start(out=res[:].rearrange("(m p) c -> p m c", p=128), in_=rsb[:])
    nc.compile()
    return nc


_PROGS = {}


def _get_progs():
    if not _PROGS:
        def mk():
            return bacc.Bacc("TRN2", target_bir_lowering=False, debug=False,
                             enable_asserts=True, num_devices=8)
        _PROGS["A"] = _build_A(mk())
        _PROGS["B"] = _build_BC(mk(), first=True)
        _PROGS["C"] = _build_BC(mk(), first=False)
        _PROGS["D1"] = _build_D1(mk())
        _PROGS["D2"] = _build_D2(mk())
    return _PROGS


def kernel(hidden_states, phi_w, psi_w, gat_lin_w, gat_att, wp, w1, b1, w2, b2,
           _profile=None):
    f32 = np.float32
    bf16 = ml_dtypes.bfloat16
    fp8 = ml_dtypes.float8_e4m3
    hidden_states = np.asarray(hidden_states, f32)
    progs = _get_progs()
    C = lambda a: np.ascontiguousarray(a)
    times = {}

    def run(tag, in_maps, core_ids):
        r = run_bass_kernel_spmd(progs[tag], in_maps, core_ids=core_ids)
        if _profile is not None:
            times[tag] = r.exec_time_ns
        return r.results

    # ---- launch A: edge build + V ----
    xT16b = [C(hidden_states[b].T.astype(bf16)) for b in range(B)]
    pwT16 = C(np.asarray(phi_w, f32).T.astype(bf16))
    swT16 = C(np.asarray(psi_w, f32).T.astype(bf16))
    glw = np.asarray(gat_lin_w, f32)
    ga = np.asarray(gat_att, f32)
    in_a = []
    for c in range(8):
        b, q = c // 4, c % 4
        l, hd = c // 4, c % 4
        Wr = glw[l, hd * H:(hd + 1) * H, :]   # [e, d]
        a2 = ga[l, hd].reshape(2, H) * WS
        a2w = a2.reshape(2, NB, 128).transpose(2, 1, 0).reshape(128, NB * 2)
        in_a.append({
            "xT16": xT16b[b],
            "xTq16": C(xT16b[b][:, q * CH:(q + 1) * CH]),
            "pwT16": pwT16, "swT16": swT16,
            "Wr8": C((Wr * WS).astype(fp8)),
            "a2w8": C(a2w.astype(fp8)),
            "srcx": C(np.arange(q * CH, (q + 1) * CH, dtype=f32)[:, None]),
        })
    ra = run("A", in_a, list(range(8)))
    m0b = [C(np.concatenate([ra[b * 4 + q]["m0o"] for q in range(4)], 0))
           for b in range(B)]
    V16s = [np.asarray(ra[c]["V16o"]) for c in range(8)]

    # ---- launches B, C ----
    prev = None
    for li, tag in enumerate(("B", "C")):
        in_l = []
        for c in range(8):
            b, hd = c // 4, c % 4
            Wm = glw[li, hd * H:(hd + 1) * H, :]
            d = {
                "WT16": C(Wm.T.astype(bf16)),
                "m0d": m0b[b],
                "V16": V16s[li * 4 + hd],
            }
            if li == 0:
                d["xT16"] = xT16b[b]
            else:
                for i in range(4):
                    d[f"p{i}"] = prev[b * 4 + i]
            in_l.append(d)
        rl = run(tag, in_l, list(range(8)))
        prev = [np.asarray(rl[c]["gT"]) for c in range(8)]

    # ---- launch D1: pooling partials (8 cores: batch x node-quarter) ----
    wpc = C(np.asarray(wp, f32).reshape(H, 1).astype(bf16))
    in_d1 = []
    for c in range(8):
        b, q = c // 4, c % 4
        d = {f"p{i}": C(prev[b * 4 + i][:, q * CH:(q + 1) * CH]) for i in range(4)}
        d["wpc"] = wpc
        in_d1.append(d)
    rd1 = run("D1", in_d1, list(range(8)))

    # ---- launch D2: combine + head (2 cores) ----
    in_d2 = []
    for b in range(B):
        uqs = np.concatenate([rd1[b * 4 + q]["uo"] for q in range(4)], 1)
        zqs = np.concatenate([rd1[b * 4 + q]["zo"] for q in range(4)], 1)
        in_d2.append({
            "uq": C(uqs.astype(f32)), "zq": C(zqs.astype(f32)),
            "w1T": C(np.asarray(w1, f32).T.astype(bf16)),
            "b1c": C(np.asarray(b1, f32)[:, None]),
            "w2T": C(np.asarray(w2, f32).T.astype(bf16)),
            "b2c": C(np.asarray(b2, f32)[:, None]),
        })
    rd2 = run("D2", in_d2, [0, 1])
    out = np.stack([rd2[b]["res"][:, 0].astype(f32) for b in range(B)])
    if _profile is not None:
        _profile.update(times)
    return out
